# revision 7
# baseline (speedup 1.0000x reference)
"""Trainium2 Bass kernel for a single attention head with query-axis softmax.

Reference semantics (per batch b):
    k = x @ Wk; q = x @ Wq; v = x @ Wv                 # [T, H]
    wei = (q @ k^T) * E**-0.5                          # [T(query), T(key)]
    wei = where(tril, wei, -inf)                       # causal: keep s <= t
    p = softmax(wei, axis=0 over query t)              # NOTE: query axis!
    out = p @ v                                        # [T, H]

Because the softmax normalizes over the query axis t (per key column s),
we can write out[t,h] = sum_s E[t,s] * v[s,h] / d[s] with
E[t,s] = exp(wei[t,s]) (masked to 0 for s > t) and d[s] = sum_t E[t,s].
The kernel computes E^T tiles ([s on partitions, t free]) so that d is a
free-axis row sum (fused into the exp instruction via accum_out), scales
v rows by 1/d, and accumulates out^T = v'^T.T-style matmuls on PE.

Sharding: batch dim (8) across the 8 NeuronCores, weights replicated.
The host passes x pre-transposed per batch (xT[b] = x[b].T) so no
on-device transpose of the big activation tensor is needed.
"""

import numpy as np

import concourse.bass as bass
import concourse.tile as tile
from concourse import bacc, mybir
from concourse import bass_utils

B, T, E, H = 8, 2048, 1024, 64
P = 128                       # partitions
CB = 512                      # column block (t) width
NE = E // P                   # 8 contraction chunks for projections
NJ = T // CB                  # 4 column blocks
NI = T // P                   # 16 s-chunks
SCALE = float(E) ** -0.5      # note: embed**-0.5, not head_size**-0.5
MASK_NEG = -1.0e30
F32 = mybir.dt.float32
F32R = mybir.dt.float32r
X = mybir.AxisListType.X
EXP = mybir.ActivationFunctionType.Exp


def _r(ap):
    """View an SBUF AP as float32r so PE runs the fast fp32 matmul path."""
    return ap.bitcast(F32R)


def _emit(tc, xT_d, wq_d, wkv_d, masks_d, ident_d, out_d):
    nc = tc.nc
    from contextlib import ExitStack

    with ExitStack() as ctx:
        singles = ctx.enter_context(tc.tile_pool(name="singles", bufs=1))
        xpool = ctx.enter_context(tc.tile_pool(name="x", bufs=16))
        epool = ctx.enter_context(tc.tile_pool(name="erow", bufs=4))
        dpool = ctx.enter_context(tc.tile_pool(name="dsmall", bufs=8))
        vpool = ctx.enter_context(tc.tile_pool(name="vrow", bufs=4))
        opool = ctx.enter_context(tc.tile_pool(name="onat", bufs=8))
        pq = ctx.enter_context(tc.tile_pool(name="pq", bufs=1, space="PSUM"))
        pkv = ctx.enter_context(tc.tile_pool(name="pkv", bufs=1, space="PSUM"))
        ps = ctx.enter_context(tc.tile_pool(name="ps", bufs=2, space="PSUM"))
        pout = ctx.enter_context(tc.tile_pool(name="pout", bufs=1, space="PSUM"))

        # --- constants -----------------------------------------------------
        ident = singles.tile([P, P], F32R)
        nc.sync.dma_start(out=ident[:], in_=ident_d[:])

        # additive causal masks for the 4 diagonal-block alignments
        masks_sb = []
        for r in range(4):
            m = singles.tile([P, CB], F32, tag=f"mask{r}", name=f"mask{r}")
            nc.sync.dma_start(out=m[:], in_=masks_d[r])
            masks_sb.append(m)

        wq_sb = singles.tile([P, NE, H], F32R)
        wkv_sb = singles.tile([P, NE, 2 * H], F32R)
        for e in range(NE):
            nc.sync.dma_start(out=wq_sb[:, e, :], in_=wq_d[e * P : (e + 1) * P, :])
            nc.sync.dma_start(out=wkv_sb[:, e, :], in_=wkv_d[e * P : (e + 1) * P, :])

        # persistent activations
        qv_sb = singles.tile([P, T], F32R)   # rows 0:64 = q^T, rows 64:128 = v^T
        kT_sb = singles.tile([H, T], F32R)   # k^T
        outT_sb = singles.tile([H, T], F32R)

        pout_tiles = [
            pout.tile([H, CB], F32, tag=f"pout{j}", name=f"pout{j}")
            for j in range(NJ)
        ]

        # --- main pipeline: column blocks in descending order --------------
        for j in reversed(range(NJ)):
            t0 = j * CB
            # load x^T column block (all NE partition chunks)
            xts = []
            for e in range(NE):
                xt = xpool.tile([P, CB], F32R)
                nc.sync.dma_start(
                    out=xt[:], in_=xT_d[e * P : (e + 1) * P, t0 : t0 + CB]
                )
                xts.append(xt)

            # q^T projection ([64, 512])
            psq = pq.tile([H, CB], F32)
            for e in range(NE):
                nc.tensor.matmul(
                    psq[:],
                    lhsT=(wq_sb[:, e, :]),
                    rhs=xts[e][:],
                    start=(e == 0),
                    stop=(e == NE - 1),
                )
            nc.vector.tensor_copy(qv_sb[0:H, t0 : t0 + CB], psq[:])

            # packed [k^T; v^T] projection ([128, 512]; k rows 0:64, v rows 64:128)
            pskv = pkv.tile([P, CB], F32)
            for e in range(NE):
                nc.tensor.matmul(
                    pskv[:],
                    lhsT=(wkv_sb[:, e, :]),
                    rhs=xts[e][:],
                    start=(e == 0),
                    stop=(e == NE - 1),
                )
            nc.vector.tensor_copy(kT_sb[:, t0 : t0 + CB], pskv[0:H, :])
            nc.vector.tensor_copy(qv_sb[H:P, t0 : t0 + CB], pskv[H:P, :])

            # rows i = 4j .. 4j+3 of E^T are now computable in full
            for r in range(4):
                i = 4 * j + r
                s0 = i * P
                nblk = NJ - j
                erow = epool.tile([P, T], F32R)
                dparts = dpool.tile([P, 4], F32, tag="dparts")
                for jj in range(j, NJ):
                    c = (jj - j) * CB
                    pst = ps.tile([P, CB], F32, tag="ps")
                    nc.tensor.matmul(
                        pst[:],
                        lhsT=(kT_sb[:, s0 : s0 + P]),
                        rhs=(qv_sb[0:H, jj * CB : (jj + 1) * CB]),
                        start=True,
                        stop=True,
                    )
                    if jj == j:
                        nc.vector.tensor_add(pst[:], pst[:], masks_sb[r][:])
                    nc.scalar.activation(
                        out=erow[:, c : c + CB],
                        in_=pst[:],
                        func=EXP,
                        scale=SCALE,
                        accum_out=dparts[:, jj - j : jj - j + 1],
                    )

                # d = sum of block sums; v'_i = v_i / d
                dinv = dpool.tile([P, 1], F32, tag="dinv")
                if nblk > 1:
                    dsum = dpool.tile([P, 1], F32, tag="dsum")
                    nc.vector.reduce_sum(dsum[:], dparts[:, 0:nblk], axis=X)
                    nc.vector.reciprocal(dinv[:], dsum[:])
                else:
                    nc.vector.reciprocal(dinv[:], dparts[:, 0:1])

                # transpose v^T slice -> v natural [128, 64], scaled by 1/d
                pvt = ps.tile([P, CB], F32R, tag="ps")
                nc.tensor.transpose(
                    pvt[:, 0:H],
                    qv_sb[H:P, s0 : s0 + P],
                    ident[H:P, H:P],
                )
                vi = vpool.tile([P, H], F32R)
                nc.vector.tensor_scalar_mul(vi[:], pvt[:, 0:H], dinv[:])

                # out^T[:, tj'] += v'_i^T-contraction with E^T row i
                for jj in range(j, NJ):
                    c = (jj - j) * CB
                    nc.tensor.matmul(
                        pout_tiles[jj][:],
                        lhsT=(vi[:]),
                        rhs=(erow[:, c : c + CB]),
                        start=(jj == j and r == 0),
                        stop=(j == 0 and r == 3),
                    )

        # --- finale: out^T -> out natural, DMA to DRAM ---------------------
        for jj in range(NJ):
            nc.vector.tensor_copy(outT_sb[:, jj * CB : (jj + 1) * CB], pout_tiles[jj][:])
        for c in range(NI):
            pso = ps.tile([P, CB], F32R, tag="ps")
            nc.tensor.transpose(
                pso[:, 0:H],
                outT_sb[:, c * P : (c + 1) * P],
                ident[0:H, 0:H],
            )
            onat = opool.tile([P, H], F32)
            nc.vector.tensor_copy(onat[:], pso[:, 0:H])
            nc.sync.dma_start(out=out_d[c * P : (c + 1) * P, :], in_=onat[:])


def _build_program():
    nc = bacc.Bacc("TRN2", target_bir_lowering=False, debug=False, num_devices=B)
    xT_d = nc.dram_tensor("xT", [E, T], F32R, kind="ExternalInput").ap()
    wq_d = nc.dram_tensor("wq", [E, H], F32R, kind="ExternalInput").ap()
    wkv_d = nc.dram_tensor("wkv", [E, 2 * H], F32R, kind="ExternalInput").ap()
    masks_d = nc.dram_tensor("masks", [4, P, CB], F32, kind="ExternalInput").ap()
    ident_d = nc.dram_tensor("ident", [P, P], F32R, kind="ExternalInput").ap()
    out_d = nc.dram_tensor("out", [T, H], F32, kind="ExternalOutput").ap()
    with tile.TileContext(nc) as tc:
        _emit(tc, xT_d, wq_d, wkv_d, masks_d, ident_d, out_d)
    nc.compile()
    return nc


def _host_masks():
    m = np.full((4, P, CB), MASK_NEG, dtype=np.float32)
    p = np.arange(P)[:, None]
    f = np.arange(CB)[None, :]
    for r in range(4):
        m[r][f >= (P * r + p)] = 0.0
    return m


def _host_inputs(x, Wk, Wq, Wv):
    x = np.asarray(x, dtype=np.float32)
    xT = np.ascontiguousarray(np.transpose(x, (0, 2, 1)))  # [B, E, T]
    wq = np.ascontiguousarray(np.asarray(Wq, dtype=np.float32))
    wkv = np.ascontiguousarray(
        np.concatenate(
            [np.asarray(Wk, dtype=np.float32), np.asarray(Wv, dtype=np.float32)],
            axis=1,
        )
    )
    masks = _host_masks()
    ident = np.eye(P, dtype=np.float32)
    return [
        {"xT": xT[b], "wq": wq, "wkv": wkv, "masks": masks, "ident": ident}
        for b in range(B)
    ]


def _ensure_axon_ntff_hook():
    """The agent image's antenv lacks axon_hooks; synthesize it so
    run_bass_kernel_spmd's trace path can find the NTFF profile hook."""
    import sys
    import types

    if "antenv.axon_hooks" in sys.modules:
        return
    try:
        import antenv

        mod = types.ModuleType("antenv.axon_hooks")
        mod._hook = None

        def set_axon_ntff_profile_hook(h):
            mod._hook = h

        def get_axon_ntff_profile_hook():
            return mod._hook

        mod.set_axon_ntff_profile_hook = set_axon_ntff_profile_hook
        mod.get_axon_ntff_profile_hook = get_axon_ntff_profile_hook
        sys.modules["antenv.axon_hooks"] = mod
        antenv.axon_hooks = mod

        from trn_agent_boot.trn_boot import _ntff_profile_via_ctypes

        hook = _ntff_profile_via_ctypes("/opt/axon/libaxon_pjrt.so")
        if hook is not None:
            mod._hook = hook
    except Exception as e:  # degrade to untraced run
        print(f"NTFF hook setup failed ({e}); tracing will be skipped")


def kernel(x, Wk, Wq, Wv, _trace=False, _trace_kwargs=None):
    if _trace:
        _ensure_axon_ntff_hook()
    in_maps = _host_inputs(x, Wk, Wq, Wv)
    nc = _build_program()
    res = bass_utils.run_bass_kernel_spmd(
        nc, in_maps, list(range(B)), trace=_trace, **(_trace_kwargs or {})
    )
    out = np.stack([res.results[b]["out"] for b in range(B)], axis=0)
    if _trace:
        kernel.last_results = res
    return out.astype(np.float32)


# revision 9
# speedup vs baseline: 1.0819x; 1.0819x over previous
"""Trainium2 Bass kernel for a single attention head with query-axis softmax.

Reference semantics (per batch b):
    k = x @ Wk; q = x @ Wq; v = x @ Wv                 # [T, H]
    wei = (q @ k^T) * E**-0.5                          # [T(query), T(key)]
    wei = where(tril, wei, -inf)                       # causal: keep s <= t
    p = softmax(wei, axis=0 over query t)              # NOTE: query axis!
    out = p @ v                                        # [T, H]

Because the softmax normalizes over the query axis t (per key column s),
out[t,h] = sum_s E[t,s] * v[s,h] / d[s] with E[t,s] = exp(wei[t,s])
(zero for s > t) and d[s] = sum_t E[t,s].  The kernel computes E^T tiles
([s on partitions, t free]) so d is a free-axis row sum (fused into the
exp instruction via accum_out), scales v rows by 1/d, and accumulates
out^T on PE.

Sharding: batch dim (8) across the 8 NeuronCores, weights replicated.
The host passes x pre-transposed per batch (xT[b] = x[b].T) in bf16 so
no on-device transpose of the big activation tensor is needed; matmul
operands are bf16 (fp32 PSUM accumulation), the v / out paths stay
fp32(r) end-to-end.
"""

import numpy as np
import ml_dtypes

import concourse.bass as bass
import concourse.tile as tile
from concourse import bacc, mybir
from concourse import bass_utils

B, T, E, H = 8, 2048, 1024, 64
P = 128                       # partitions
CB = 512                      # column block (t) width
NE = E // P                   # 8 contraction chunks for projections
NJ = T // CB                  # 4 column blocks
NI = T // P                   # 16 s-chunks
SCALE = float(E) ** -0.5      # note: embed**-0.5, not head_size**-0.5
MASK_NEG = -1.0e30
F32 = mybir.dt.float32
F32R = mybir.dt.float32r
BF16 = mybir.dt.bfloat16
X = mybir.AxisListType.X
EXP = mybir.ActivationFunctionType.Exp


def _emit(tc, xT_d, wq_d, wkv_d, masks_d, identr_d, out_d):
    nc = tc.nc
    from contextlib import ExitStack

    with ExitStack() as ctx:
        singles = ctx.enter_context(tc.tile_pool(name="singles", bufs=1))
        xpool = ctx.enter_context(tc.tile_pool(name="x", bufs=16))
        epool = ctx.enter_context(tc.tile_pool(name="erow", bufs=4))
        dpool = ctx.enter_context(tc.tile_pool(name="dsmall", bufs=8))
        vpool = ctx.enter_context(tc.tile_pool(name="vrow", bufs=4))
        opool = ctx.enter_context(tc.tile_pool(name="onat", bufs=8))
        ps = ctx.enter_context(tc.tile_pool(name="ps", bufs=2, space="PSUM"))
        pout = ctx.enter_context(tc.tile_pool(name="pout", bufs=1, space="PSUM"))

        # --- constants (host-packed, single DMAs) --------------------------
        identr = singles.tile([P, P], F32R)
        nc.sync.dma_start(out=identr[:], in_=identr_d[:])
        # 4 additive causal masks, packed [128, 4*512] (mask r at cols 512r)
        masks_sb = singles.tile([P, 4 * CB], F32)
        nc.sync.dma_start(out=masks_sb[:], in_=masks_d[:])
        # weights host-packed: wq [128, 8*64], wkv [128, 8*128]
        wq_sb = singles.tile([P, NE * H], BF16)
        nc.sync.dma_start(out=wq_sb[:], in_=wq_d[:])
        wkv_sb = singles.tile([P, NE * 2 * H], BF16)
        nc.sync.dma_start(out=wkv_sb[:], in_=wkv_d[:])

        # persistent activations
        q_sb = singles.tile([H, T], BF16)    # q^T
        kT_sb = singles.tile([H, T], BF16)   # k^T
        vT_sb = singles.tile([P, T], F32R)   # v^T lives in rows 64:128
        outT_sb = singles.tile([H, T], F32R)

        # out^T accumulators: one [64, 512] bank per column block
        pout_tiles = [
            pout.tile([H, CB], F32, tag=f"pt{a}", name=f"pt{a}") for a in range(NJ)
        ]

        def pout_slice(jj, c0, c1):
            return pout_tiles[jj][:, c0:c1]

        # x^T resident tiles: 16 DMAs of [128, 1024] bf16, high half first
        xts = [[None] * 2 for _ in range(NE)]
        for half in (1, 0):
            for e in range(NE):
                xt = xpool.tile([P, T // 2], BF16, tag="xt", name=f"xt{e}_{half}")
                nc.sync.dma_start(
                    out=xt[:],
                    in_=xT_d[
                        e * P : (e + 1) * P, half * (T // 2) : (half + 1) * (T // 2)
                    ],
                )
                xts[e][half] = xt

        def x_rhs(e, j):
            half = j // 2
            c0 = (j - 2 * half) * CB
            return xts[e][half][:, c0 : c0 + CB]

        # --- main pipeline: column blocks in descending order --------------
        for j in reversed(range(NJ)):
            t0 = j * CB
            # projections: q^T in rows 0:64 of bank A, [k^T; v^T] in bank B
            pproj = ps.tile([P, 2 * CB], F32, tag="ps", name="pproj")
            for e in range(NE):
                nc.tensor.matmul(
                    pproj[0:H, 0:CB],
                    lhsT=wq_sb[:, e * H : (e + 1) * H],
                    rhs=x_rhs(e, j),
                    start=(e == 0),
                    stop=(e == NE - 1),
                )
            for e in range(NE):
                nc.tensor.matmul(
                    pproj[:, CB : 2 * CB],
                    lhsT=wkv_sb[:, e * 2 * H : (e + 1) * 2 * H],
                    rhs=x_rhs(e, j),
                    start=(e == 0),
                    stop=(e == NE - 1),
                )
            nc.vector.tensor_copy(q_sb[:, t0 : t0 + CB], pproj[0:H, 0:CB])
            nc.vector.tensor_copy(kT_sb[:, t0 : t0 + CB], pproj[0:H, CB : 2 * CB])
            nc.vector.tensor_copy(vT_sb[H:P, t0 : t0 + CB], pproj[H:P, CB : 2 * CB])

            # rows i = 4j .. 4j+3 of E^T are now computable in full
            for r in range(4):
                i = 4 * j + r
                s0 = i * P
                nblk = NJ - j
                d0 = r * P  # first unmasked column of the diagonal block
                erow = epool.tile([P, T], BF16)
                dparts = dpool.tile([P, 2], F32, tag="dparts")
                npair = (nblk + 1) // 2
                for pair in range(npair):
                    jj0 = j + 2 * pair
                    w = CB * min(2, NJ - jj0)  # 512 or 1024
                    pst = ps.tile([P, 2 * CB], F32, tag="ps")
                    for u in range(w // CB):
                        jj = jj0 + u
                        nc.tensor.matmul(
                            pst[:, u * CB : (u + 1) * CB],
                            lhsT=kT_sb[:, s0 : s0 + P],
                            rhs=q_sb[:, jj * CB : (jj + 1) * CB],
                            start=True,
                            stop=True,
                        )
                    lo = d0 if pair == 0 else 0  # skip fully-masked diag cols
                    if pair == 0 and lo < CB:
                        nc.vector.tensor_add(
                            pst[:, lo:CB],
                            pst[:, lo:CB],
                            masks_sb[:, r * CB + lo : (r + 1) * CB],
                        )
                    c = 2 * CB * pair
                    nc.scalar.activation(
                        out=erow[:, c + lo : c + w],
                        in_=pst[:, lo:w],
                        func=EXP,
                        scale=SCALE,
                        accum_out=dparts[:, pair : pair + 1],
                    )

                # d = sum of block sums; v'_i = v_i / d
                dinv = dpool.tile([P, 1], F32, tag="dinv")
                if npair > 1:
                    dsum = dpool.tile([P, 1], F32, tag="dsum")
                    nc.vector.reduce_sum(dsum[:], dparts[:, 0:npair], axis=X)
                    nc.vector.reciprocal(dinv[:], dsum[:])
                else:
                    nc.vector.reciprocal(dinv[:], dparts[:, 0:1])

                # transpose v^T slice -> v natural [128, 64], scaled by 1/d
                pvt = ps.tile([P, 2 * CB], F32R, tag="ps")
                nc.tensor.transpose(
                    pvt[:, 0:H],
                    vT_sb[H:P, s0 : s0 + P],
                    identr[H:P, H:P],
                )
                vi = vpool.tile([P, H], BF16)
                nc.vector.tensor_scalar_mul(vi[:], pvt[:, 0:H], dinv[:])

                # out^T[:, tj'] += v'_i-contraction with E^T row i
                for jj in range(j, NJ):
                    c = (jj - j) * CB
                    lo = d0 if jj == j else 0
                    nc.tensor.matmul(
                        pout_slice(jj, lo, CB),
                        lhsT=vi[:],
                        rhs=erow[:, c + lo : c + CB],
                        start=(jj == j and r == 0),
                        stop=(j == 0 and r == 3),
                    )

        # --- finale: out^T -> out natural, DMA to DRAM ---------------------
        for a in range(NJ):
            nc.vector.tensor_copy(
                outT_sb[:, a * CB : (a + 1) * CB], pout_tiles[a][:]
            )
        for c in range(NI):
            pso = ps.tile([P, 2 * CB], F32R, tag="ps")
            nc.tensor.transpose(
                pso[:, 0:H],
                outT_sb[:, c * P : (c + 1) * P],
                identr[0:H, 0:H],
            )
            onat = opool.tile([P, H], F32)
            nc.vector.tensor_copy(onat[:], pso[:, 0:H])
            nc.sync.dma_start(out=out_d[c * P : (c + 1) * P, :], in_=onat[:])


def _build_program():
    nc = bacc.Bacc("TRN2", target_bir_lowering=False, debug=False, num_devices=B)
    xT_d = nc.dram_tensor("xT", [E, T], BF16, kind="ExternalInput").ap()
    wq_d = nc.dram_tensor("wq", [P, NE * H], BF16, kind="ExternalInput").ap()
    wkv_d = nc.dram_tensor("wkv", [P, NE * 2 * H], BF16, kind="ExternalInput").ap()
    masks_d = nc.dram_tensor("masks", [P, 4 * CB], F32, kind="ExternalInput").ap()
    identr_d = nc.dram_tensor("identr", [P, P], F32R, kind="ExternalInput").ap()
    out_d = nc.dram_tensor("out", [T, H], F32, kind="ExternalOutput").ap()
    with tile.TileContext(nc) as tc:
        _emit(tc, xT_d, wq_d, wkv_d, masks_d, identr_d, out_d)
    nc.compile()
    return nc


def _host_masks():
    """Packed [128, 4*512]: mask r at cols [512r, 512r+512)."""
    m = np.full((P, 4 * CB), MASK_NEG, dtype=np.float32)
    p = np.arange(P)[:, None]
    f = np.arange(CB)[None, :]
    for r in range(4):
        m[:, r * CB : (r + 1) * CB][f >= (P * r + p)] = 0.0
    return m


def _host_inputs(x, Wk, Wq, Wv):
    bf = ml_dtypes.bfloat16
    x = np.asarray(x, dtype=np.float32)
    xT = np.ascontiguousarray(np.transpose(x, (0, 2, 1))).astype(bf)  # [B, E, T]

    def pack_w(*ws):
        # [E, h_tot] (concat) -> [128, NE * h_tot]: chunk e at cols e*h_tot
        w = np.concatenate([np.asarray(a, np.float32) for a in ws], axis=1)
        h = w.shape[1]
        return np.ascontiguousarray(
            w.reshape(NE, P, h).transpose(1, 0, 2).reshape(P, NE * h)
        ).astype(bf)

    wq = pack_w(Wq)
    wkv = pack_w(Wk, Wv)
    masks = _host_masks()
    ident = np.eye(P, dtype=np.float32)
    return [
        {"xT": xT[b], "wq": wq, "wkv": wkv, "masks": masks, "identr": ident}
        for b in range(B)
    ]


def _ensure_axon_ntff_hook():
    """The agent image's antenv lacks axon_hooks; synthesize it so
    run_bass_kernel_spmd's trace path can find the NTFF profile hook."""
    import sys
    import types

    if "antenv.axon_hooks" in sys.modules:
        return
    try:
        import antenv

        mod = types.ModuleType("antenv.axon_hooks")
        mod._hook = None

        def set_axon_ntff_profile_hook(h):
            mod._hook = h

        def get_axon_ntff_profile_hook():
            return mod._hook

        mod.set_axon_ntff_profile_hook = set_axon_ntff_profile_hook
        mod.get_axon_ntff_profile_hook = get_axon_ntff_profile_hook
        sys.modules["antenv.axon_hooks"] = mod
        antenv.axon_hooks = mod

        from trn_agent_boot.trn_boot import _ntff_profile_via_ctypes

        hook = _ntff_profile_via_ctypes("/opt/axon/libaxon_pjrt.so")
        if hook is not None:
            mod._hook = hook
    except Exception as e:  # degrade to untraced run
        print(f"NTFF hook setup failed ({e}); tracing will be skipped")


def kernel(x, Wk, Wq, Wv, _trace=False, _trace_kwargs=None):
    if _trace:
        _ensure_axon_ntff_hook()
    in_maps = _host_inputs(x, Wk, Wq, Wv)
    nc = _build_program()
    res = bass_utils.run_bass_kernel_spmd(
        nc, in_maps, list(range(B)), trace=_trace, **(_trace_kwargs or {})
    )
    out = np.stack([res.results[b]["out"] for b in range(B)], axis=0)
    if _trace:
        kernel.last_results = res
    return out.astype(np.float32)


# revision 10
# speedup vs baseline: 1.2499x; 1.1552x over previous
"""Trainium2 Bass kernel for a single attention head with query-axis softmax.

Reference semantics (per batch b):
    k = x @ Wk; q = x @ Wq; v = x @ Wv                 # [T, H]
    wei = (q @ k^T) * E**-0.5                          # [T(query), T(key)]
    wei = where(tril, wei, -inf)                       # causal: keep s <= t
    p = softmax(wei, axis=0 over query t)              # NOTE: query axis!
    out = p @ v                                        # [T, H]

Because the softmax normalizes over the query axis t (per key column s),
out[t,h] = sum_s E[t,s] * v[s,h] / d[s] with E[t,s] = exp(wei[t,s])
(zero for s > t) and d[s] = sum_t E[t,s].  The kernel computes E^T tiles
([s on partitions, t free]) so d is a free-axis row sum (fused into the
exp instruction via accum_out), scales v rows by 1/d, and accumulates
out^T on PE.

Sharding: batch dim (8) across the 8 NeuronCores, weights replicated.
The host passes x pre-transposed per batch (xT[b] = x[b].T) in bf16 so
no on-device transpose of the big activation tensor is needed; matmul
operands are bf16 (fp32 PSUM accumulation), the v / out paths stay
fp32(r) end-to-end.
"""

import numpy as np
import ml_dtypes

import concourse.bass as bass
import concourse.tile as tile
from concourse import bacc, mybir
from concourse import bass_utils

B, T, E, H = 8, 2048, 1024, 64
P = 128                       # partitions
CB = 512                      # column block (t) width
NE = E // P                   # 8 contraction chunks for projections
NJ = T // CB                  # 4 column blocks
NI = T // P                   # 16 s-chunks
SCALE = float(E) ** -0.5      # note: embed**-0.5, not head_size**-0.5
MASK_NEG = -1.0e30
F32 = mybir.dt.float32
F32R = mybir.dt.float32r
BF16 = mybir.dt.bfloat16
X = mybir.AxisListType.X
EXP = mybir.ActivationFunctionType.Exp


def _emit(tc, xT_d, wq_d, wkv_d, masks_d, identr_d, out_d):
    nc = tc.nc
    from contextlib import ExitStack

    with ExitStack() as ctx:
        singles = ctx.enter_context(tc.tile_pool(name="singles", bufs=1))
        xpool = ctx.enter_context(tc.tile_pool(name="x", bufs=16))
        epool = ctx.enter_context(tc.tile_pool(name="erow", bufs=6))
        dpool = ctx.enter_context(tc.tile_pool(name="dsmall", bufs=8))
        vpool = ctx.enter_context(tc.tile_pool(name="vrow", bufs=4))
        opool = ctx.enter_context(tc.tile_pool(name="onat", bufs=8))
        ps = ctx.enter_context(tc.tile_pool(name="ps", bufs=2, space="PSUM"))
        pproj_pool = ctx.enter_context(tc.tile_pool(name="pproj", bufs=1, space="PSUM"))
        pout = ctx.enter_context(tc.tile_pool(name="pout", bufs=1, space="PSUM"))

        # --- constants (host-packed, single DMAs) --------------------------
        identr = singles.tile([P, P], F32R)
        nc.sync.dma_start(out=identr[:], in_=identr_d[:])
        # 4 additive causal masks, packed [128, 4*512] (mask r at cols 512r)
        masks_sb = singles.tile([P, 4 * CB], F32)
        nc.sync.dma_start(out=masks_sb[:], in_=masks_d[:])
        # weights host-packed: wq [128, 8*64], wkv [128, 8*128]
        wq_sb = singles.tile([P, NE * H], BF16)
        nc.sync.dma_start(out=wq_sb[:], in_=wq_d[:])
        wkv_sb = singles.tile([P, NE * 2 * H], BF16)
        nc.sync.dma_start(out=wkv_sb[:], in_=wkv_d[:])

        # persistent activations
        q_sb = singles.tile([H, T], BF16)    # q^T
        kT_sb = singles.tile([H, T], BF16)   # k^T
        vT_sb = singles.tile([P, T], F32R)   # v^T lives in rows 64:128
        outT_sb = singles.tile([P, T // 2], F32R)  # rows 0:64 jj even, 64:128 odd

        # out^T accumulators packed 2 per bank: jj even rows 0:64, odd 64:128.
        # Accumulation groups on disjoint partition ranges of one bank are
        # fine on HW (per-element has_written); skip the sim's coarse check.
        pout_tiles = [
            pout.tile([P, CB], F32, tag=f"pt{a}", name=f"pt{a}") for a in range(2)
        ]

        def pout_slice(jj, c0, c1):
            rb = H * (jj % 2)
            return pout_tiles[jj // 2][rb : rb + H, c0:c1]

        # x^T resident tiles: j=3 column first (small DMAs so PE starts
        # early), then the j=0..2 columns as one wide DMA per e-chunk
        xts3 = []
        for e in range(NE):
            xt = xpool.tile([P, CB], BF16, tag="xt3", name=f"xt3_{e}")
            nc.sync.dma_start(out=xt[:], in_=xT_d[e * P : (e + 1) * P, 3 * CB :])
            xts3.append(xt)
        xtsr = []
        for e in range(NE):
            xt = xpool.tile([P, 3 * CB], BF16, tag="xtr", name=f"xtr_{e}")
            nc.sync.dma_start(out=xt[:], in_=xT_d[e * P : (e + 1) * P, 0 : 3 * CB])
            xtsr.append(xt)

        def x_rhs(e, j):
            if j == 3:
                return xts3[e][:]
            return xtsr[e][:, j * CB : (j + 1) * CB]

        # --- main pipeline: column blocks in descending order --------------
        for j in reversed(range(NJ)):
            t0 = j * CB
            # projections: q^T in rows 0:64 of bank A, [k^T; v^T] in bank B
            pproj = pproj_pool.tile([P, 2 * CB], F32, tag="pp", name="pproj")
            for e in range(NE):
                nc.tensor.matmul(
                    pproj[0:H, 0:CB],
                    lhsT=wq_sb[:, e * H : (e + 1) * H],
                    rhs=x_rhs(e, j),
                    start=(e == 0),
                    stop=(e == NE - 1),
                )
            for e in range(NE):
                nc.tensor.matmul(
                    pproj[:, CB : 2 * CB],
                    lhsT=wkv_sb[:, e * 2 * H : (e + 1) * 2 * H],
                    rhs=x_rhs(e, j),
                    start=(e == 0),
                    stop=(e == NE - 1),
                )
            nc.vector.tensor_copy(q_sb[:, t0 : t0 + CB], pproj[0:H, 0:CB])
            nc.vector.tensor_copy(kT_sb[:, t0 : t0 + CB], pproj[0:H, CB : 2 * CB])
            nc.vector.tensor_copy(vT_sb[H:P, t0 : t0 + CB], pproj[H:P, CB : 2 * CB])

            # rows i = 4j .. 4j+3 of E^T are now computable in full
            for r in range(4):
                i = 4 * j + r
                s0 = i * P
                nblk = NJ - j
                d0 = r * P  # first unmasked column of the diagonal block
                erow = epool.tile([P, T], BF16)
                dparts = dpool.tile([P, 2], F32, tag="dparts")
                npair = (nblk + 1) // 2
                for pair in range(npair):
                    jj0 = j + 2 * pair
                    w = CB * min(2, NJ - jj0)  # 512 or 1024
                    pst = ps.tile([P, 2 * CB], F32, tag="ps")
                    for u in range(w // CB):
                        jj = jj0 + u
                        nc.tensor.matmul(
                            pst[:, u * CB : (u + 1) * CB],
                            lhsT=kT_sb[:, s0 : s0 + P],
                            rhs=q_sb[:, jj * CB : (jj + 1) * CB],
                            start=True,
                            stop=True,
                        )
                    lo = d0 if pair == 0 else 0  # skip fully-masked diag cols
                    if pair == 0 and lo < CB:
                        nc.vector.tensor_add(
                            pst[:, lo:CB],
                            pst[:, lo:CB],
                            masks_sb[:, r * CB + lo : (r + 1) * CB],
                        )
                    c = 2 * CB * pair
                    nc.scalar.activation(
                        out=erow[:, c + lo : c + w],
                        in_=pst[:, lo:w],
                        func=EXP,
                        scale=SCALE,
                        accum_out=dparts[:, pair : pair + 1],
                    )

                # d = sum of block sums; v'_i = v_i / d
                dinv = dpool.tile([P, 1], F32, tag="dinv")
                if npair > 1:
                    dsum = dpool.tile([P, 1], F32, tag="dsum")
                    nc.vector.reduce_sum(dsum[:], dparts[:, 0:npair], axis=X)
                    nc.vector.reciprocal(dinv[:], dsum[:])
                else:
                    nc.vector.reciprocal(dinv[:], dparts[:, 0:1])

                # transpose v^T slice -> v natural [128, 64], scaled by 1/d
                pvt = ps.tile([P, 2 * CB], F32R, tag="ps")
                nc.tensor.transpose(
                    pvt[:, 0:H],
                    vT_sb[H:P, s0 : s0 + P],
                    identr[H:P, H:P],
                )
                vi = vpool.tile([P, H], BF16)
                nc.vector.tensor_scalar_mul(vi[:], pvt[:, 0:H], dinv[:])

                # out^T[:, tj'] += v'_i-contraction with E^T row i
                for jj in range(j, NJ):
                    c = (jj - j) * CB
                    lo = d0 if jj == j else 0
                    nc.tensor.matmul(
                        pout_slice(jj, lo, CB),
                        lhsT=vi[:],
                        rhs=erow[:, c + lo : c + CB],
                        start=(jj == j and r == 0),
                        stop=(j == 0 and r == 3),
                        skip_group_check=True,
                    )

        # --- finale: out^T -> out natural, DMA to DRAM ---------------------
        for a in range(2):
            nc.vector.tensor_copy(
                outT_sb[:, a * CB : (a + 1) * CB], pout_tiles[a][:]
            )
        for c in range(NI):
            jj = c // 4
            rb = H * (jj % 2)
            col = (jj // 2) * CB + (c % 4) * P
            pso = ps.tile([P, 2 * CB], F32R, tag="ps")
            nc.tensor.transpose(
                pso[:, 0:H],
                outT_sb[rb : rb + H, col : col + P],
                identr[rb : rb + H, rb : rb + H],
            )
            onat = opool.tile([P, H], F32)
            nc.vector.tensor_copy(onat[:], pso[:, 0:H])
            nc.sync.dma_start(out=out_d[c * P : (c + 1) * P, :], in_=onat[:])


def _build_program():
    nc = bacc.Bacc("TRN2", target_bir_lowering=False, debug=False, num_devices=B)
    xT_d = nc.dram_tensor("xT", [E, T], BF16, kind="ExternalInput").ap()
    wq_d = nc.dram_tensor("wq", [P, NE * H], BF16, kind="ExternalInput").ap()
    wkv_d = nc.dram_tensor("wkv", [P, NE * 2 * H], BF16, kind="ExternalInput").ap()
    masks_d = nc.dram_tensor("masks", [P, 4 * CB], F32, kind="ExternalInput").ap()
    identr_d = nc.dram_tensor("identr", [P, P], F32R, kind="ExternalInput").ap()
    out_d = nc.dram_tensor("out", [T, H], F32, kind="ExternalOutput").ap()
    with tile.TileContext(nc) as tc:
        _emit(tc, xT_d, wq_d, wkv_d, masks_d, identr_d, out_d)
    nc.compile()
    return nc


def _host_masks():
    """Packed [128, 4*512]: mask r at cols [512r, 512r+512)."""
    m = np.full((P, 4 * CB), MASK_NEG, dtype=np.float32)
    p = np.arange(P)[:, None]
    f = np.arange(CB)[None, :]
    for r in range(4):
        m[:, r * CB : (r + 1) * CB][f >= (P * r + p)] = 0.0
    return m


def _host_inputs(x, Wk, Wq, Wv):
    bf = ml_dtypes.bfloat16
    x = np.asarray(x, dtype=np.float32)
    xT = np.ascontiguousarray(np.transpose(x, (0, 2, 1))).astype(bf)  # [B, E, T]

    def pack_w(*ws):
        # [E, h_tot] (concat) -> [128, NE * h_tot]: chunk e at cols e*h_tot
        w = np.concatenate([np.asarray(a, np.float32) for a in ws], axis=1)
        h = w.shape[1]
        return np.ascontiguousarray(
            w.reshape(NE, P, h).transpose(1, 0, 2).reshape(P, NE * h)
        ).astype(bf)

    wq = pack_w(Wq)
    wkv = pack_w(Wk, Wv)
    masks = _host_masks()
    ident = np.eye(P, dtype=np.float32)
    return [
        {"xT": xT[b], "wq": wq, "wkv": wkv, "masks": masks, "identr": ident}
        for b in range(B)
    ]


def _ensure_axon_ntff_hook():
    """The agent image's antenv lacks axon_hooks; synthesize it so
    run_bass_kernel_spmd's trace path can find the NTFF profile hook."""
    import sys
    import types

    if "antenv.axon_hooks" in sys.modules:
        return
    try:
        import antenv

        mod = types.ModuleType("antenv.axon_hooks")
        mod._hook = None

        def set_axon_ntff_profile_hook(h):
            mod._hook = h

        def get_axon_ntff_profile_hook():
            return mod._hook

        mod.set_axon_ntff_profile_hook = set_axon_ntff_profile_hook
        mod.get_axon_ntff_profile_hook = get_axon_ntff_profile_hook
        sys.modules["antenv.axon_hooks"] = mod
        antenv.axon_hooks = mod

        from trn_agent_boot.trn_boot import _ntff_profile_via_ctypes

        hook = _ntff_profile_via_ctypes("/opt/axon/libaxon_pjrt.so")
        if hook is not None:
            mod._hook = hook
    except Exception as e:  # degrade to untraced run
        print(f"NTFF hook setup failed ({e}); tracing will be skipped")


def kernel(x, Wk, Wq, Wv, _trace=False, _trace_kwargs=None):
    if _trace:
        _ensure_axon_ntff_hook()
    in_maps = _host_inputs(x, Wk, Wq, Wv)
    nc = _build_program()
    res = bass_utils.run_bass_kernel_spmd(
        nc, in_maps, list(range(B)), trace=_trace, **(_trace_kwargs or {})
    )
    out = np.stack([res.results[b]["out"] for b in range(B)], axis=0)
    if _trace:
        kernel.last_results = res
    return out.astype(np.float32)
